# revision 3
# baseline (speedup 1.0000x reference)
"""Fused RoPE attention + LayerNorm, Trainium2, 8 NeuronCores (SPMD).

Head-sharded (tensor parallel): each core computes 2 of the 16 heads for
ALL 4096 tokens.  Inputs are broadcast (HBM reads, no link traffic); the
only communication is a per-batch bf16 AllToAll of the attention output
(0.5 MB per core per batch) followed by a row-sharded LayerNorm.

Per-core layouts:
  Q^T, K^T  [128 = 2 heads x 64 dh, 4096 tok]   (RoPE'd, bf16)
  V         [128 kpos, 32 blocks x (64|1|64|1)] (ones col -> denominator)
  scores    [128 kpos, q] psum -> exp on ScalarE -> pt bf16
  attn      [128 q, 65] = pt^T @ [V|1]  (psum; col 64 = sum of exp)

Scores matmuls are 2-way row-tiled (tile_position (0,0)/(64,0)): both
heads' K=64 matmuls run concurrently in the two halves of the PE array.
"""
import sys
import types
import os
import numpy as np
from contextlib import ExitStack

for _p in ("/opt/trn_rl_repo",):
    if _p not in sys.path:
        sys.path.append(_p)

if "antenv.axon_hooks" not in sys.modules:
    _hooks = types.ModuleType("antenv.axon_hooks")
    _HOOK = [None]
    _hooks.set_axon_ntff_profile_hook = lambda h: _HOOK.__setitem__(0, h)
    _hooks.get_axon_ntff_profile_hook = lambda: _HOOK[0]
    sys.modules["antenv.axon_hooks"] = _hooks
    try:
        from trn_agent_boot.trn_boot import _ntff_profile_via_ctypes

        _HOOK[0] = _ntff_profile_via_ctypes("/opt/axon/libaxon_pjrt.so")
    except Exception:
        pass

import concourse.bass as bass  # noqa: E402
import concourse.bacc as bacc  # noqa: E402
import concourse.mybir as mybir  # noqa: E402
import concourse.tile as tile  # noqa: E402
from concourse import bass_utils  # noqa: E402

F32 = mybir.dt.float32
BF16 = mybir.dt.bfloat16
NP_BF16 = np.dtype(mybir.dt.np(BF16))
AF = mybir.ActivationFunctionType
ALU = mybir.AluOpType
AX = mybir.AxisListType

B, S, D, H, DH = 2, 2048, 1024, 16, 64
NC = 8
TOK = B * S            # 4096
DC = D // 128          # 8 contraction chunks
KT = S // 128          # 16 k-tiles per batch
NBW = 512              # token block width (proj + q-block)
NB = S // NBW          # 4 blocks per batch
SLOTW = 512            # score slot width (q cols)
SPT = 2                # slots per psum/exp tile  -> [128, 1024]
LN_EPS = 1e-5
ROPE_BASE = 10000.0
RG = [list(range(NC))]


def _build(flags):
    has_bqk, has_bv, has_gb = flags
    ROWTILE = os.environ.get("KV2_ROWTILE", "1") == "1"
    LNRSTD = os.environ.get("KV2_LNRSTD", "1") == "1"
    STAGE = int(os.environ.get("KV2_STAGE", "4"))
    nc = bacc.Bacc("TRN2", target_bir_lowering=False, debug=False,
                   num_devices=NC)

    xqT_d = nc.dram_tensor("xqT", [D, TOK], BF16, kind="ExternalInput")
    xvT_d = nc.dram_tensor("xvT", [D, TOK], BF16, kind="ExternalInput")
    wq_d = nc.dram_tensor("wq", [D, 128], BF16, kind="ExternalInput")
    wk_d = nc.dram_tensor("wk", [D, 128], BF16, kind="ExternalInput")
    wv_d = nc.dram_tensor("wv", [D, 128], BF16, kind="ExternalInput")
    ident_d = nc.dram_tensor("ident", [128, 128], BF16,
                             kind="ExternalInput")
    perm_d = nc.dram_tensor("perm", [128, 128], BF16, kind="ExternalInput")
    cos_d = nc.dram_tensor("cos", [128, TOK], BF16, kind="ExternalInput")
    sin_d = nc.dram_tensor("sin", [128, TOK], BF16, kind="ExternalInput")
    if has_bqk:
        cq_d = nc.dram_tensor("cq", [128, TOK], F32, kind="ExternalInput")
        ck_d = nc.dram_tensor("ck", [128, TOK], F32, kind="ExternalInput")
    if has_bv:
        bv_d = nc.dram_tensor("bv", [128, 128], F32, kind="ExternalInput")
    if has_gb:
        gam_d = nc.dram_tensor("gamma", [128, D], F32, kind="ExternalInput")
        bet_d = nc.dram_tensor("beta", [128, D], F32, kind="ExternalInput")
    out_d = nc.dram_tensor("out", [2 * 256, D], F32, kind="ExternalOutput")

    es = ExitStack()
    with es:
        tc = es.enter_context(tile.TileContext(nc))
        dram = es.enter_context(tc.tile_pool(name="dram", bufs=1,
                                             space="DRAM"))
        constp = es.enter_context(tc.tile_pool(name="const", bufs=1))
        qkp = es.enter_context(tc.tile_pool(name="qkp", bufs=1))
        vp = es.enter_context(tc.tile_pool(name="vp", bufs=1))
        xp = es.enter_context(tc.tile_pool(name="xp", bufs=2))
        xvp = es.enter_context(tc.tile_pool(name="xvp", bufs=2))
        usp = es.enter_context(tc.tile_pool(name="usp", bufs=2))
        tst = es.enter_context(tc.tile_pool(name="tst", bufs=4))
        ptp = es.enter_context(tc.tile_pool(name="ptp", bufs=24))
        aop = es.enter_context(tc.tile_pool(name="aop", bufs=4))
        epi = es.enter_context(tc.tile_pool(name="epi", bufs=10))
        lnp = es.enter_context(tc.tile_pool(name="lnp", bufs=4))
        lsqp = es.enter_context(tc.tile_pool(name="lsqp", bufs=2))
        lop = es.enter_context(tc.tile_pool(name="lop", bufs=4))
        atp = es.enter_context(tc.tile_pool(name="atp", bufs=2))
        # psum: pj 1 + pv 1 + pst 2x2 + paT 1 + ptr 1 = 8 banks
        pj = es.enter_context(tc.tile_pool(name="pj", bufs=1, space="PSUM"))
        pv = es.enter_context(tc.tile_pool(name="pv", bufs=1, space="PSUM"))
        pst = es.enter_context(tc.tile_pool(name="pst", bufs=2,
                                            space="PSUM"))
        paT = es.enter_context(tc.tile_pool(name="paT", bufs=1,
                                            space="PSUM"))
        ptr = es.enter_context(tc.tile_pool(name="ptr", bufs=1,
                                            space="PSUM"))

        bounce = [dram.tile([S, 128], BF16, tag=f"bn{b}", name=f"bn{b}")
                  for b in range(B)]
        a2a = [dram.tile([S, 128], BF16, tag=f"a2a{b}", name=f"a2a{b}")
               for b in range(B)]

        perm_sb = constp.tile([128, 128], BF16, tag="perm")
        ident_sb = constp.tile([128, 128], BF16, tag="ident")
        cos_sb = constp.tile([128, TOK], BF16, tag="cos")
        sin_sb = constp.tile([128, TOK], BF16, tag="sin")
        wq_sb = constp.tile([128, DC * 128], BF16, tag="wq")
        wk_sb = constp.tile([128, DC * 128], BF16, tag="wk")
        wv_sb = constp.tile([128, DC * 128], BF16, tag="wv")
        U32 = mybir.dt.uint32
        magic_sb = constp.tile([128, 1], U32, tag="magic")
        nc.vector.memset(magic_sb[:], 0x5F3759DF)
        one_u = constp.tile([128, 1], U32, tag="oneu")
        nc.vector.memset(one_u[:], 1)
        cq_sb = ck_sb = bv_sb = gam_sb = bet_sb = None
        if has_bqk:
            cq_sb = constp.tile([128, TOK], F32, tag="cq")
            ck_sb = constp.tile([128, TOK], F32, tag="ck")
            nc.sync.dma_start(cq_sb[:], cq_d[:])
            nc.sync.dma_start(ck_sb[:], ck_d[:])
        if has_bv:
            bv_sb = constp.tile([128, 128], F32, tag="bvs")
            nc.sync.dma_start(bv_sb[:], bv_d[:])
        if has_gb:
            gam_sb = constp.tile([128, D], F32, tag="gam")
            nc.sync.dma_start(gam_sb[:], gam_d[:])
            bet_sb = constp.tile([128, D], F32, tag="bet")
            nc.sync.dma_start(bet_sb[:], bet_d[:])

        def load_w(dst_sb, t_dram):
            nc.sync.dma_start(
                dst_sb[:].rearrange("p (c d) -> p c d", d=128),
                t_dram[:].rearrange("(c p) d -> p c d", p=128))

        # critical-path order: wq/wk + first x pieces first (each one DMA)
        load_w(wq_sb, wq_d)
        load_w(wk_sb, wk_d)

        q_sb = qkp.tile([128, TOK], BF16, tag="q")
        k_sb = qkp.tile([128, TOK], BF16, tag="k")
        q1_sb = k1_sb = None
        if not ROWTILE:
            q1_sb = qkp.tile([64, TOK], BF16, tag="q1")
            k1_sb = qkp.tile([64, TOK], BF16, tag="k1")
        # V blocks: (b, kt) -> [64 dh h0 | 1 | 64 dh h1 | 1]
        v_sb = vp.tile([128, B * KT * 130], BF16, tag="v")
        v65 = v_sb[:].rearrange("p (t e) -> p t e", e=65)
        nc.vector.memset(v65[:, :, 64:65], 1.0)

        eps_sb = constp.tile([128, 1], F32, tag="eps")
        nc.vector.memset(eps_sb[:], LN_EPS)

        # warm the (single) activation table set early
        warm = epi.tile([128, 1], F32, tag="warm")
        nc.vector.memset(warm[:], 1.0)
        warm2 = epi.tile([128, 1], F32, tag="warm2")
        nc.scalar.activation(warm2[:], warm[:], AF.Exp)

        # ---------------- projections ----------------
        def load_x(pool, t_dram, b, i, nm):
            t = pool.tile([128, DC * NBW], BF16, tag="x", name=nm)
            c0 = b * S + i * NBW
            src = t_dram[:].rearrange("(c p) t -> p c t", p=128)
            nc.sync.dma_start(t[:].rearrange("p (c w) -> p c w", w=NBW),
                              src[:, :, c0:c0 + NBW])
            return t

        xq00 = load_x(xp, xqT_d, 0, 0, "xq00")
        xv00 = load_x(xvp, xvT_d, 0, 0, "xv00")
        nc.sync.dma_start(perm_sb[:], perm_d[:])
        nc.sync.dma_start(cos_sb[:], cos_d[:])
        nc.sync.dma_start(sin_sb[:], sin_d[:])
        load_w(wv_sb, wv_d)
        nc.sync.dma_start(ident_sb[:], ident_d[:])

        def proj_qk(w_sb, x_t, dst, col0, c_sb, nm, mul_eng):
            # psum is read only by the two Vector copies; RoPE math runs
            # on GpSimd from the bf16 SBUF copies (GpSimd can't read PSUM)
            ps = pj.tile([128, NBW], F32, tag="pj", name=f"ps{nm}")
            for dc in range(DC):
                nc.tensor.matmul(ps[:],
                                 w_sb[:, dc * 128:(dc + 1) * 128],
                                 x_t[:, dc * NBW:(dc + 1) * NBW],
                                 start=(dc == 0), stop=(dc == DC - 1))
            u = usp.tile([128, NBW], BF16, tag="u", name=f"u{nm}")
            nc.vector.tensor_copy(u[:], ps[:])
            ps2 = pv.tile([128, NBW], F32, tag="pv", name=f"ps2{nm}")
            nc.tensor.matmul(ps2[:], perm_sb[:], u[:], start=True, stop=True)
            u2 = usp.tile([128, NBW], BF16, tag="u", name=f"u2{nm}")
            nc.vector.tensor_copy(u2[:], ps2[:])
            t1 = tst.tile([128, NBW], F32, tag="t", name=f"t1{nm}")
            mul_eng.tensor_tensor(t1[:], u[:], cos_sb[:, col0:col0 + NBW],
                                  ALU.mult)
            t2 = tst.tile([128, NBW], F32, tag="t", name=f"t2{nm}")
            mul_eng.tensor_tensor(t2[:], u2[:], sin_sb[:, col0:col0 + NBW],
                                  ALU.mult)
            if c_sb is None:
                nc.vector.tensor_tensor(dst[:, col0:col0 + NBW], t1[:],
                                        t2[:], ALU.add)
            else:
                t3 = tst.tile([128, NBW], F32, tag="t", name=f"t3{nm}")
                nc.vector.tensor_tensor(t3[:], t1[:], t2[:], ALU.add)
                nc.vector.tensor_tensor(dst[:, col0:col0 + NBW], t3[:],
                                        c_sb[:, col0:col0 + NBW], ALU.add)

        def proj_v(x_t, b, i):
            ps = pv.tile([128, NBW], F32, tag="pv", name=f"pv{b}_{i}")
            for st in range(4):
                sl = ps[:, st * 128:(st + 1) * 128]
                for dc in range(DC):
                    nc.tensor.matmul(
                        sl,
                        x_t[:, dc * NBW + st * 128: dc * NBW + st * 128 + 128],
                        wv_sb[:, dc * 128:(dc + 1) * 128],
                        start=(dc == 0), stop=(dc == DC - 1))
                if has_bv:
                    nc.vector.tensor_tensor(sl, sl, bv_sb[:], ALU.add)
                blk = (b * KT + i * 4 + st) * 130
                nc.vector.tensor_copy(v_sb[:, blk:blk + 64],
                                      ps[:, st * 128:st * 128 + 64])
                nc.vector.tensor_copy(v_sb[:, blk + 65:blk + 129],
                                      ps[:, st * 128 + 64:st * 128 + 128])

        # ---------------- scores + exp ----------------
        ptmap = {}

        class ScoreGroup:
            def __init__(self, b, qb):
                self.b = b
                self.qb = qb
                self.ns = 0
                self.tile = None
                self.tidx = 0

            def add(self, kt):
                b, qb = self.b, self.qb
                for h in range(2):
                    if self.tile is None:
                        self.tile = pst.tile(
                            [128, SPT * SLOTW], F32, tag="pst",
                            name=f"pst{b}_{qb}_{self.tidx}")
                        self.fill = 0
                    s = self.fill
                    if ROWTILE:
                        ksl = k_sb[h * 64:(h + 1) * 64,
                                   b * S + kt * 128: b * S + (kt + 1) * 128]
                        qsl = q_sb[h * 64:(h + 1) * 64,
                                   b * S + qb * NBW: b * S + (qb + 1) * NBW]
                        tp = (h * 64, 0)
                    else:
                        ksrc = k_sb if h == 0 else k1_sb
                        qsrc = q_sb if h == 0 else q1_sb
                        ksl = ksrc[0:64,
                                   b * S + kt * 128: b * S + (kt + 1) * 128]
                        qsl = qsrc[0:64,
                                   b * S + qb * NBW: b * S + (qb + 1) * NBW]
                        tp = None
                    nc.tensor.matmul(
                        self.tile[:, s * SLOTW:(s + 1) * SLOTW],
                        ksl, qsl, start=True, stop=True, tile_position=tp)
                    ptmap[(b, qb, kt, h)] = (self.tidx, s)
                    self.fill += 1
                    self.ns += 1
                    if self.fill == SPT:
                        self.flush()

            def flush(self):
                if self.tile is None:
                    return
                n = self.fill
                ptt = ptp.tile([128, SPT * SLOTW], BF16, tag="pt",
                               name=f"pt{self.b}_{self.qb}_{self.tidx}")
                nc.scalar.activation(ptt[:, 0:n * SLOTW],
                                     self.tile[:, 0:n * SLOTW],
                                     AF.Exp, scale=0.125)
                self.pts = getattr(self, "pts", {})
                self.pts[self.tidx] = ptt
                self.tile = None
                self.tidx += 1

        groups = {}

        def scores_add(b, qb, kts):
            g = groups.get((b, qb))
            if g is None:
                g = groups[(b, qb)] = ScoreGroup(b, qb)
            for kt in kts:
                g.add(kt)
            if g.ns == 2 * KT:
                g.flush()

        # ---------------- AV + epilogue ----------------
        # attn^T accumulation: aT[65, 512] = [V_h|1]^T @ P^T with V as the
        # stationary operand (65-col weight loads, N=512 matmuls), then
        # PE-transpose back to [q, dh] and normalize by the ones-row.
        aomap = {}

        def av_h(b, qb, h):
            g = groups[(b, qb)]
            aT = paT.tile([65, NBW], F32, tag="aT", name=f"aT{b}_{qb}_{h}")
            for kt in range(KT):
                ti, s = ptmap[(b, qb, kt, h)]
                nc.tensor.matmul(
                    aT[:],
                    v_sb[:, (b * KT + kt) * 130 + h * 65:
                         (b * KT + kt) * 130 + (h + 1) * 65],
                    g.pts[ti][:, s * SLOTW:(s + 1) * SLOTW],
                    start=(kt == 0), stop=(kt == KT - 1))
            aT_sb = atp.tile([65, NBW], BF16, tag="ats",
                             name=f"ats{b}_{qb}_{h}")
            nc.vector.tensor_copy(aT_sb[:], aT[:])
            tr = ptr.tile([128, 4 * 66], BF16, tag="tr",
                          name=f"tr{b}_{qb}_{h}")
            for t in range(4):
                nc.tensor.transpose(tr[:, t * 66:t * 66 + 65],
                                    aT_sb[:, t * 128:(t + 1) * 128],
                                    ident_sb[0:65, 0:65])
            rec = epi.tile([128, 4], F32, tag="rec",
                           name=f"rec{b}_{qb}_{h}")
            nc.vector.reciprocal(rec[:], tr[:, 64::66])
            for t in range(4):
                key = (b, qb, t)
                if h == 0:
                    aomap[key] = aop.tile([128, 128], BF16, tag="ao",
                                          name=f"ao{b}_{qb}_{t}")
                ao = aomap[key]
                nc.vector.tensor_scalar(
                    ao[:, h * 64:(h + 1) * 64],
                    tr[:, t * 66:t * 66 + 64],
                    rec[:, t:t + 1], None, ALU.mult)
                if h == 1:
                    qtg = qb * 4 + t
                    nc.sync.dma_start(
                        bounce[b][qtg * 128:(qtg + 1) * 128, :], ao[:])

        def av_qb(b, qb):
            av_h(b, qb, 0)
            av_h(b, qb, 1)

        # ---------------- LayerNorm (table-free, any vector engine) ------
        def layer_norm(b, eng):
            for t in range(2):
                li = lnp.tile([128, D], BF16, tag="li", name=f"li{b}_{t}")
                src = a2a[b][:].rearrange("(i r) c -> r i c", r=256)
                nc.sync.dma_start(
                    li[:].rearrange("p (i c) -> p i c", c=128),
                    src[t * 128:(t + 1) * 128, :, :])
                sums = epi.tile([128, 1], F32, tag="s1", name=f"s1_{b}{t}")
                nc.vector.reduce_sum(sums[:], li[:], axis=AX.X)
                sq = lsqp.tile([128, D], F32, tag="sq", name=f"sq{b}{t}")
                ssum = epi.tile([128, 1], F32, tag="s2", name=f"s2_{b}{t}")
                nc.scalar.activation(sq[:], li[:], AF.Square,
                                     accum_out=ssum[:])
                mu = epi.tile([128, 1], F32, tag="mu", name=f"mu{b}{t}")
                eng.tensor_scalar_mul(mu[:], sums[:], 1.0 / D)
                var = epi.tile([128, 1], F32, tag="va", name=f"va{b}{t}")
                eng.tensor_scalar(var[:], mu[:], mu[:], None, ALU.mult)
                eng.scalar_tensor_tensor(
                    var[:], ssum[:], 1.0 / D, var[:], ALU.mult,
                    ALU.subtract)
                # table-free rsqrt: bit-hack seed + 2 Newton steps
                ve = epi.tile([128, 1], F32, tag="ve", name=f"ve{b}{t}")
                eng.tensor_scalar_add(ve[:], var[:], LN_EPS)
                sh = epi.tile([128, 1], F32, tag="sh", name=f"sh{b}{t}")
                eng.tensor_tensor(sh[:].bitcast(U32),
                                  ve[:].bitcast(U32), one_u[:],
                                  ALU.logical_shift_right)
                y0 = epi.tile([128, 1], F32, tag="y0", name=f"y0{b}{t}")
                eng.tensor_tensor(y0[:].bitcast(U32), magic_sb[:],
                                  sh[:].bitcast(U32), ALU.subtract)
                for it in range(2):
                    a = epi.tile([128, 1], F32, tag="nt",
                                 name=f"nt{b}{t}{it}")
                    eng.tensor_tensor(a[:], y0[:], y0[:], ALU.mult)
                    eng.tensor_tensor(a[:], a[:], ve[:], ALU.mult)
                    eng.tensor_scalar(a[:], a[:], -0.5, 1.5,
                                      ALU.mult, ALU.add)
                    eng.tensor_tensor(y0[:], y0[:], a[:], ALU.mult)
                rstd = y0
                mrs = epi.tile([128, 1], F32, tag="mr", name=f"mr{b}{t}")
                eng.tensor_tensor(mrs[:], mu[:], rstd[:], ALU.mult)
                o = lop.tile([128, D], F32, tag="o", name=f"o{b}{t}")
                eng.tensor_scalar(o[:], li[:], rstd[:], mrs[:],
                                  ALU.mult, ALU.subtract)
                if has_gb:
                    eng.tensor_tensor(o[:], o[:], gam_sb[:], ALU.mult)
                    eng.tensor_tensor(o[:], o[:], bet_sb[:], ALU.add)
                nc.sync.dma_start(out_d[b * 256 + t * 128:
                                        b * 256 + (t + 1) * 128, :], o[:])

        # ---------------- emission schedule ----------------
        def phase1_i(b, i):
            if b == 0 and i == 0:
                xq_t = xq00
            else:
                xq_t = load_x(xp, xqT_d, b, i, f"xq{b}_{i}")
            col0 = b * S + i * NBW
            meng = nc.gpsimd
            proj_qk(wq_sb, xq_t, q_sb, col0, cq_sb, f"q{b}{i}", meng)
            proj_qk(wk_sb, xq_t, k_sb, col0, ck_sb, f"k{b}{i}", meng)
            if not ROWTILE:
                nc.sync.dma_start(q1_sb[:, col0:col0 + NBW],
                                  q_sb[64:128, col0:col0 + NBW])
                nc.sync.dma_start(k1_sb[:, col0:col0 + NBW],
                                  k_sb[64:128, col0:col0 + NBW])
            xv_t = xv00 if (b == 0 and i == 0) else \
                load_x(xvp, xvT_d, b, i, f"xv{b}_{i}")
            proj_v(xv_t, b, i)
            scores_add(b, 0, range(4 * i, 4 * i + 4))

        def phase1(b, mid=None):
            for i in range(NB):
                if mid is not None and i == 1:
                    mid()
                phase1_i(b, i)

        def phase2(b, noav=False, mid=None, per_j=None):
            for j in range(1, NB):
                for kt in range(KT):
                    scores_add(b, j, [kt])
                    if not noav:
                        if kt == 7:
                            av_h(b, j - 1, 0)
                        elif kt == 15:
                            av_h(b, j - 1, 1)
                if per_j is not None and j - 1 < len(per_j):
                    per_j[j - 1]()
                if mid is not None and j == 2:
                    mid()

        def dummy_out():
            oz = lop.tile([128, D], F32, tag="oz", name="oz")
            nc.vector.memset(oz[:], 0.0)
            for r in range(4):
                nc.sync.dma_start(out_d[r * 128:(r + 1) * 128, :], oz[:])

        if STAGE == 1:
            # projections only; dump q_sb so nothing is dead
            phase1(0)
            phase1(1)
            oz = lop.tile([128, D], F32, tag="oz", name="oz")
            nc.vector.tensor_copy(oz[:], q_sb[:, 0:D])
            nc.vector.tensor_add(oz[:], oz[:], k_sb[:, 0:D])
            nc.vector.tensor_add(oz[:], oz[:], v_sb[:, 0:D])
            for r in range(4):
                nc.sync.dma_start(out_d[r * 128:(r + 1) * 128, :], oz[:])
        elif STAGE == 2:
            # + scores/exp (pts consumed into a reduction dump)
            phase1(0)
            phase2(0, noav=True)
            phase1(1)
            phase2(1, noav=True)
            oz = lop.tile([128, D], F32, tag="oz", name="oz")
            acc = epi.tile([128, 1], F32, tag="acc", name="acc")
            g = groups[(1, 3)]
            nc.vector.reduce_sum(acc[:], g.pts[0][:], axis=AX.X)
            nc.vector.memset(oz[:], 0.0)
            nc.vector.tensor_scalar(oz[:, 0:1], acc[:], 1.0, None, ALU.mult)
            for r in range(4):
                nc.sync.dma_start(out_d[r * 128:(r + 1) * 128, :], oz[:])
        elif STAGE == 3:
            # + AV + bounce writes, no collective / LN
            phase1(0)
            phase2(0)
            phase1(1)
            av_qb(0, 3)
            phase2(1)
            av_qb(1, 3)
            dummy_out()
        else:
            phase1(0)
            phase2(0)
            phase1(1)
            av_qb(0, 3)
            nc.gpsimd.collective_compute(
                "AllToAll", ALU.bypass,
                ins=[bounce[0][:].opt()], outs=[a2a[0][:].opt()],
                replica_groups=RG)
            phase2(1)
            av_h(1, 3, 0)
            av_h(1, 3, 1)
            layer_norm(0, nc.vector)
            nc.gpsimd.collective_compute(
                "AllToAll", ALU.bypass,
                ins=[bounce[1][:].opt()], outs=[a2a[1][:].opt()],
                replica_groups=RG)
            layer_norm(1, nc.vector)

    nc.compile()
    return nc


_CACHE: dict = {}
LAST_EXEC_NS = None


def _rope_tables():
    half = DH // 2
    inv_freq = 1.0 / (ROPE_BASE ** (np.arange(half, dtype=np.float32) / half))
    t = np.arange(S, dtype=np.float32)
    freqs = t[:, None] * inv_freq[None, :]
    emb = np.concatenate([freqs, freqs], axis=-1)          # [S, DH]
    return np.cos(emb).astype(np.float32), np.sin(emb).astype(np.float32)


def _perm_mat():
    Pm = np.zeros((128, 128), np.float32)
    for i in range(64):
        Pm[2 * i + 1, 2 * i] = -1.0
        Pm[2 * i, 2 * i + 1] = 1.0
    return Pm


def prep_flags(inputs):
    b_qk = np.asarray(inputs["b_qk"], dtype=np.float32)
    b_v = np.asarray(inputs["b_v"], dtype=np.float32)
    gamma = np.asarray(inputs["ln_gamma"], dtype=np.float32)
    beta = np.asarray(inputs["ln_beta"], dtype=np.float32)
    return (bool(np.any(b_qk)), bool(np.any(b_v)),
            bool(np.any(gamma != 1.0) or np.any(beta != 0.0)))


def _prep_in_maps(inputs, flags):
    x_qk = np.asarray(inputs["x_qk"], dtype=np.float32)
    x_v = np.asarray(inputs["x_v"], dtype=np.float32)
    W_qk = np.asarray(inputs["W_qk"], dtype=np.float32)
    b_qk = np.asarray(inputs["b_qk"], dtype=np.float32)
    W_v = np.asarray(inputs["W_v"], dtype=np.float32)
    b_v = np.asarray(inputs["b_v"], dtype=np.float32)
    gamma = np.asarray(inputs["ln_gamma"], dtype=np.float32)
    beta = np.asarray(inputs["ln_beta"], dtype=np.float32)

    cos_all, sin_all = _rope_tables()                      # [S, DH]
    cos_t = np.tile(cos_all.T, (2, 2))                     # [128, TOK]
    sin_t = np.tile(sin_all.T, (2, 2))
    Pm = _perm_mat()

    xqT = np.ascontiguousarray(x_qk.reshape(TOK, D).T.astype(NP_BF16))
    xvT = np.ascontiguousarray(x_v.reshape(TOK, D).T.astype(NP_BF16))
    cos_b = np.ascontiguousarray(cos_t.astype(NP_BF16))
    sin_b = np.ascontiguousarray(sin_t.astype(NP_BF16))
    perm_b = np.ascontiguousarray(Pm.astype(NP_BF16))

    in_maps = []
    for c in range(NC):
        sl = slice(128 * c, 128 * (c + 1))
        m = {
            "xqT": xqT, "xvT": xvT,
            "wq": np.ascontiguousarray(W_qk[:, :D][:, sl].astype(NP_BF16)),
            "wk": np.ascontiguousarray(W_qk[:, D:][:, sl].astype(NP_BF16)),
            "wv": np.ascontiguousarray(W_v[:, sl].astype(NP_BF16)),
            "perm": perm_b, "cos": cos_b, "sin": sin_b,
            "ident": np.ascontiguousarray(np.eye(128, dtype=NP_BF16)),
        }
        if flags[0]:
            bq_c = b_qk[:D][sl]
            bk_c = b_qk[D:][sl]
            bq2 = Pm.T @ bq_c
            bk2 = Pm.T @ bk_c
            m["cq"] = np.ascontiguousarray(
                bq_c[:, None] * cos_t + bq2[:, None] * sin_t)
            m["ck"] = np.ascontiguousarray(
                bk_c[:, None] * cos_t + bk2[:, None] * sin_t)
        if flags[1]:
            m["bv"] = np.ascontiguousarray(
                np.broadcast_to(b_v[sl], (128, 128)).astype(np.float32))
        if flags[2]:
            m["gamma"] = np.ascontiguousarray(
                np.broadcast_to(gamma, (128, D)).astype(np.float32))
            m["beta"] = np.ascontiguousarray(
                np.broadcast_to(beta, (128, D)).astype(np.float32))
        in_maps.append(m)
    return in_maps


def assemble_output(per_core_outs):
    out = np.empty((B, S, D), np.float32)
    for c in range(NC):
        oc = np.asarray(per_core_outs[c], dtype=np.float32)
        out[0, 256 * c:256 * (c + 1)] = oc[0:256]
        out[1, 256 * c:256 * (c + 1)] = oc[256:512]
    return out


def kernel(**inputs):
    flags = prep_flags(inputs)
    if flags not in _CACHE:
        _CACHE[flags] = _build(flags)
    nc = _CACHE[flags]
    in_maps = _prep_in_maps(inputs, flags)
    res = bass_utils.run_bass_kernel_spmd(
        nc, in_maps, core_ids=list(range(NC)))
    global LAST_EXEC_NS
    LAST_EXEC_NS = res.exec_time_ns
    return assemble_output([res.results[c]["out"] for c in range(NC)])
